# revision 2
# baseline (speedup 1.0000x reference)
"""GCN encoder (2-layer GCNConv) as a Bass/Tile kernel on 8 Trainium2 NeuronCores.

Strategy (matches the sharding hint):
  - Nodes row-partitioned across 8 cores (6250 rows each); weights replicated.
  - Symmetric normalization factorized: z = D^-1/2 (A+I) D^-1/2 (x W) + b
    =>  u = dinv * (x W);  agg[d] = u[d] + sum_{e:dst=d} u[src_e];
        z = dinv * agg + b
    so no per-edge norm gather is needed.
  - Per layer: local matmul -> row scale -> AllGather(u) -> per-core gather of
    source rows (dma_gather) -> segment-sum via tensor-engine matmuls with
    compile-time-structured 0/1 selection matrices generated on DVE
    (is_equal against an iota) -> scale/bias/relu -> output rows.
  - Edges are bucketed host-side by (dst window of 128, src range) and padded
    to 128-slot tiles; padded slots gather row 0 and have an all-zero
    selection column, so they contribute nothing.

Perf notes vs the first working version (2.22 ms):
  - dma_gather descriptor generation runs on ONE Q7 core pair selected by
    queue_num; with num_swdge_queues=4 and calls spread over queues 0..3,
    four pairs generate descriptors concurrently (the dominant cost at
    ~9.3 ns per gathered row per pair; HW-verified 3x on a microbenchmark).
    queue_num is assigned AFTER scheduling (BIR position % 4) because
    Tile's SWDGE completion-semaphore lanes are assigned round-robin over
    the scheduled order and a lane must only ever be incremented by one
    queue (decoder shadow-sem rule); build-time round-robin is insufficient
    because the scheduler reorders independent gathers.
  - Each layer's AllGather is split into two window-aligned node-range
    collectives (a = first SPLITW windows, b = rest); the range-a collective
    is triggered as soon as the first half of the layer matmul finishes,
    overlapping the collective with the rest of the compute.  Each range
    also stays int16-addressable after cross-rank concat, removing a
    separate src-half split.
  - u stores to DRAM are batched into one DMA per range (rearranged AP)
    instead of one per window; large index/selector loads are issued after
    phase A so they don't delay its x-chunk streaming on the Sync queue.
  - Selection matrices for all tiles of a (window, range) group are generated
    by a single DVE is_equal over a [128, t, 128] broadcast AP.
  - Slot padding gathers row 0 (index 0): the SWDGE decoder reserves ring
    space from num_idxs_reg while the Q7 generator trims trailing -1 indices,
    so a -1-padded call leaks ring space and eventually deadlocks
    await_space.  Padding with 0 keeps both sides consistent (the Q7 loop is
    vectorized per 128-chunk, so sub-tile trimming saves no time anyway).
"""

import math
import os
import sys

import numpy as np

sys.path.insert(0, "/opt/trn_rl_repo")

import ml_dtypes

BF16 = ml_dtypes.bfloat16

NQ = int(os.environ.get("GCN_NQ", "4"))  # SWDGE queues used round-robin


class Cfg:
    def __init__(self, N, E, IN=512, HID=256, OUT=128, P=8):
        self.N, self.E, self.IN, self.HID, self.OUT, self.P = N, E, IN, HID, OUT, P
        self.NC = N // P                      # nodes per core
        self.WS = 128                         # dst window size
        self.NW = math.ceil(self.NC / self.WS)  # windows per core
        # split local rows into two window-aligned ranges; each range's
        # cross-rank concat must stay int16-addressable (< 32768 rows)
        self.SPLITW = (self.NW + 1) // 2
        self.NCA = min(self.SPLITW * self.WS, self.NC)
        self.NCB = self.NC - self.NCA
        assert P * self.NCA <= 32767 and P * self.NCB <= 32767


FULL = Cfg(N=50000, E=800000)


def _prepare(cfg, x, edge_index, W1, b1, W2, b2):
    """Host-side graph preprocessing -> per-core input maps + program params."""
    N, P, NC, WS, NW = cfg.N, cfg.P, cfg.NC, cfg.WS, cfg.NW
    NCA, NCB = cfg.NCA, cfg.NCB
    src = np.asarray(edge_index[0], dtype=np.int64)
    dst = np.asarray(edge_index[1], dtype=np.int64)

    deg = np.bincount(dst, minlength=N).astype(np.float64) + 1.0  # + self loop
    dinv = (1.0 / np.sqrt(deg)).astype(np.float32)

    # src row ids inside the two all-gathered range tensors
    s_own = src // NC
    s_loc = src % NC
    half = (s_loc >= NCA).astype(np.int64)
    s_row = np.where(half == 0, s_own * NCA + s_loc,
                     s_own * NCB + (s_loc - NCA))

    # group id: ((core, window), src-range) ; groups contiguous after sort
    win_id = (dst // NC) * NW + (dst % NC) // WS
    comp = win_id * 2 + half
    order = np.argsort(comp, kind="stable")
    r_s, d_s, c_s = s_row[order], dst[order], comp[order]
    counts = np.bincount(c_s, minlength=P * NW * 2).reshape(P, NW, 2)

    # shared tile counts per (window, range): max over cores
    T = np.ceil(counts.max(axis=0) / 128).astype(np.int64)  # [NW, 2]
    tiles_total = int(T.sum())
    slots_total = tiles_total * 128

    starts = np.zeros(P * NW * 2 + 1, dtype=np.int64)
    np.cumsum(counts.reshape(-1), out=starts[1:])

    in_maps = []
    for c in range(P):
        idx_arr = np.zeros(slots_total, dtype=np.int16)
        aco_arr = np.full(slots_total, -1, dtype=np.float32)
        off = 0
        for w in range(NW):
            for h in range(2):
                t_wh = int(T[w, h])
                if t_wh == 0:
                    continue
                g = (c * NW + w) * 2 + h
                n = counts[c, w, h]
                sl = slice(starts[g], starts[g] + n)
                idx_arr[off:off + n] = r_s[sl].astype(np.int16)
                aco_arr[off:off + n] = (d_s[sl] - c * NC - w * WS).astype(np.float32)
                off += 128 * t_wh
        assert off == slots_total

        dloc = np.concatenate(
            [dinv[c * NC:(c + 1) * NC],
             np.ones(NW * WS - NC, dtype=np.float32)])

        m = {
            "xT": np.ascontiguousarray(
                np.asarray(x[c * NC:(c + 1) * NC], np.float32).astype(BF16).T
                .reshape(cfg.IN // 128, 128, NC).transpose(1, 0, 2)),
            "w1": np.ascontiguousarray(
                np.asarray(W1, np.float32).astype(BF16)
                .reshape(cfg.IN // 128, 128, cfg.HID).transpose(1, 0, 2)),
            "w2": np.ascontiguousarray(
                np.asarray(W2, np.float32).astype(BF16)
                .reshape(cfg.HID // 128, 128, cfg.OUT).transpose(1, 0, 2)),
            "dinvc": np.ascontiguousarray(dloc.reshape(NW, WS).T),
            "idx": np.ascontiguousarray(np.tile(idx_arr.reshape(-1, 16).T, (8, 1))),
            "acol": np.ascontiguousarray(aco_arr.reshape(-1, 128).T.astype(BF16)),
            "ident": np.eye(128, dtype=BF16),
        }
        b1nz = bool(np.any(np.asarray(b1)))
        b2nz = bool(np.any(np.asarray(b2)))
        if b1nz:
            m["b1bc"] = np.ascontiguousarray(
                np.broadcast_to(np.asarray(b1, np.float32), (128, cfg.HID)))
        if b2nz:
            m["b2bc"] = np.ascontiguousarray(
                np.broadcast_to(np.asarray(b2, np.float32), (128, cfg.OUT)))
        in_maps.append(m)

    return in_maps, T, b1nz, b2nz


def build_program(cfg, T, b1nz, b2nz):
    import concourse.bacc as bacc
    import concourse.mybir as mybir
    from concourse import tile

    N, P, NC, WS, NW = cfg.N, cfg.P, cfg.NC, cfg.WS, cfg.NW
    NCA, NCB, SPLITW = cfg.NCA, cfg.NCB, cfg.SPLITW
    IN, HID, OUT = cfg.IN, cfg.HID, cfg.OUT
    NCI, NCH = IN // 128, HID // 128
    tiles_total = int(T.sum())
    slots_total = tiles_total * 128
    TMAX = int(T.max())
    f32, bf16, i16 = mybir.dt.float32, mybir.dt.bfloat16, mybir.dt.int16
    AF = mybir.ActivationFunctionType

    nc = bacc.Bacc("TRN2", target_bir_lowering=False, debug=False,
                   num_devices=cfg.P, num_swdge_queues=NQ,
                   dynamic_dma_scratch_size=int(os.environ.get(
                       "GCN_DMA_SCRATCH", "32768")))
    xT_p = nc.dram_tensor("xT", [128, NCI, NC], bf16, kind="ExternalInput")
    w1_p = nc.dram_tensor("w1", [128, NCI, HID], bf16, kind="ExternalInput")
    w2_p = nc.dram_tensor("w2", [128, NCH, OUT], bf16, kind="ExternalInput")
    dinv_p = nc.dram_tensor("dinvc", [WS, NW], f32, kind="ExternalInput")
    idx_p = nc.dram_tensor("idx", [128, slots_total // 16], i16, kind="ExternalInput")
    acol_p = nc.dram_tensor("acol", [128, tiles_total], bf16, kind="ExternalInput")
    id_p = nc.dram_tensor("ident", [128, 128], bf16, kind="ExternalInput")
    b1_p = (nc.dram_tensor("b1bc", [128, HID], f32, kind="ExternalInput")
            if b1nz else None)
    b2_p = (nc.dram_tensor("b2bc", [128, OUT], f32, kind="ExternalInput")
            if b2nz else None)
    out_p = nc.dram_tensor("out", [NC, OUT], f32, kind="ExternalOutput")

    u1da = nc.dram_tensor("u1da", [NCA, HID], bf16)
    u1db = nc.dram_tensor("u1db", [NCB, HID], bf16)
    u2da = nc.dram_tensor("u2da", [NCA, OUT], bf16)
    u2db = nc.dram_tensor("u2db", [NCB, OUT], bf16)
    U1a = nc.dram_tensor("U1a", [P * NCA, HID], bf16, addr_space="Shared")
    U1b = nc.dram_tensor("U1b", [P * NCB, HID], bf16, addr_space="Shared")
    U2a = nc.dram_tensor("U2a", [P * NCA, OUT], bf16, addr_space="Shared")
    U2b = nc.dram_tensor("U2b", [P * NCB, OUT], bf16, addr_space="Shared")
    rg = [list(range(P))]

    with tile.TileContext(nc) as tc:
        with (
            tc.tile_pool(name="res", bufs=1) as res,
            tc.tile_pool(name="work", bufs=4) as work,
            tc.tile_pool(name="gath", bufs=12) as gath,
            tc.tile_pool(name="psum", bufs=2, space="PSUM") as psum,
        ):
            # ---- resident loads needed by phase A ----
            w1s = res.tile([128, NCI, HID], bf16)
            nc.sync.dma_start(w1s[:], w1_p[:])
            w2s = res.tile([128, NCH, OUT], bf16)
            nc.sync.dma_start(w2s[:], w2_p[:])
            dinvs = res.tile([WS, NW], f32)
            nc.sync.dma_start(dinvs[:], dinv_p[:])
            ident = res.tile([128, 128], bf16)
            nc.sync.dma_start(ident[:], id_p[:])
            iot = res.tile([128, TMAX, 128], bf16)
            nc.gpsimd.iota(iot[:], pattern=[[0, TMAX], [1, 128]], base=0,
                           channel_multiplier=0,
                           allow_small_or_imprecise_dtypes=True)
            b1bc = None
            if b1nz:
                b1bc = res.tile([128, HID], f32)
                nc.sync.dma_start(b1bc[:], b1_p[:])
            b2bc = None
            if b2nz:
                b2bc = res.tile([128, OUT], f32)
                nc.sync.dma_start(b2bc[:], b2_p[:])

            # hoist gather-count registers (one per distinct tile count) so
            # each dma_gather doesn't spend a Pool MOVE slot on its count
            cnt_regs = {}
            for t_wh in sorted(set(int(t) for t in T.reshape(-1) if t)):
                cnt_regs[t_wh] = nc.gpsimd.to_reg(128 * t_wh)

            u1res = res.tile([128, NW, HID], bf16)
            u2res = res.tile([128, NW, OUT], bf16)
            h1T = res.tile([128, NCH, NC], bf16)
            idxs = res.tile([128, slots_total // 16], i16)
            acols = res.tile([128, tiles_total], bf16)
            if NC % WS:
                # tail rows of the last window feed the self-loop matmul as
                # rhs; zero them so uninitialized SBUF can't inject NaNs
                nc.gpsimd.memset(u1res[:, NW - 1, :], 0.0)
                nc.gpsimd.memset(u2res[:, NW - 1, :], 0.0)

            def nsz(j):
                return min(128, NC - j * WS)

            MAXP = int(os.environ.get("GCN_MAX_PHASE", "9"))

            def emit_debug_out(src_bf16_ap, w, n):
                # convert [n, OUT] bf16 -> f32, dump into out rows of window w
                dt = work.tile([128, OUT], f32, tag="dbg")
                nc.scalar.activation(dt[:n, :], src_bf16_ap, AF.Copy)
                nc.sync.dma_start(out_p[w * WS:w * WS + n, :], dt[:n, :])

            def store_rng(ud, ures, rng_a):
                """Batched store of a window range of ures into ud."""
                if rng_a:
                    w0, rows = 0, NCA
                else:
                    w0, rows = SPLITW, NCB
                nfull = rows // WS
                tail = rows - nfull * WS
                if nfull:
                    dst = ud[0:nfull * WS, :].rearrange(
                        "(w p) f -> p w f", p=WS)
                    nc.sync.dma_start(dst, ures[:, w0:w0 + nfull, :])
                if tail:
                    nc.sync.dma_start(ud[nfull * WS:, :],
                                      ures[:tail, w0 + nfull, :])

            # ---- phase A: t1 = x @ W1 ; u1 = dinv * t1 ; split AllGather ----
            for j in range(NW):
                n = nsz(j)
                jsl = slice(j * WS, j * WS + n)
                xc = work.tile([128, NCI, WS], bf16, tag="xc")
                nc.sync.dma_start(xc[:, :, :n], xT_p[:, :, jsl])
                pt = psum.tile([128, HID], f32, tag="mm")
                for ci in range(NCI):
                    nc.tensor.matmul(pt[:n, :], xc[:, ci, :n],
                                     w1s[:, ci, :], start=(ci == 0),
                                     stop=(ci == NCI - 1))
                nc.scalar.activation(u1res[:n, j, :], pt[:n, :], AF.Copy,
                                     scale=dinvs[:n, j:j + 1])
                if MAXP == 1:
                    emit_debug_out(u1res[:n, j, :OUT], j, n)
                if j == SPLITW - 1:
                    store_rng(u1da, u1res, True)
                    nc.gpsimd.collective_compute(
                        "AllGather", mybir.AluOpType.bypass,
                        replica_groups=rg, ins=[u1da[:]], outs=[U1a[:]])
            store_rng(u1db, u1res, False)
            if MAXP <= 1:
                return nc
            nc.gpsimd.collective_compute(
                "AllGather", mybir.AluOpType.bypass, replica_groups=rg,
                ins=[u1db[:]], outs=[U1b[:]])

            # big constant loads deferred here so they don't delay phase A's
            # x-chunk streaming on the Sync DMA queue
            nc.sync.dma_start(idxs[:], idx_p[:])
            nc.sync.dma_start(acols[:], acol_p[:])

            # ---- generic aggregation layer ----
            def agg_layer(Ua, Ub, F, ures, bbc, relu, emit_out):
                tile_idx = 0
                slot_off = 0
                call_no = 0
                for w in range(NW):
                    n = nsz(w)
                    pa = psum.tile([128, F], f32, tag="agg")
                    # self-loop term: ident.T @ u[w]
                    nc.tensor.matmul(pa[:n, :], ident[:, :n], ures[:, w, :],
                                     start=True, stop=False)
                    nmm = int(T[w, 0] + T[w, 1])
                    done = 0
                    for h in range(2):
                        t_wh = int(T[w, h])
                        if t_wh == 0:
                            continue
                        q = call_no % NQ
                        call_no += 1
                        g = gath.tile([128, TMAX, F], bf16, tag="g")
                        U = Ua if h == 0 else Ub
                        nc.gpsimd.dma_gather(
                            g[:, :t_wh, :], U[:],
                            idxs[:, slot_off // 16:
                                 (slot_off + 128 * t_wh) // 16],
                            num_idxs=128 * t_wh, num_idxs_reg=cnt_regs[t_wh],
                            elem_size=F, single_packet=False, queue_num=q)
                        slot_off += 128 * t_wh
                        S = work.tile([128, TMAX, 128], bf16, tag="S")
                        nc.vector.tensor_tensor(
                            S[:, :t_wh, :], iot[:, :t_wh, :],
                            acols[:, tile_idx:tile_idx + t_wh]
                            .broadcast_to((128, t_wh, 128)),
                            op=mybir.AluOpType.is_equal)
                        tile_idx += t_wh
                        for t in range(t_wh):
                            done += 1
                            nc.tensor.matmul(pa[:n, :], S[:, t, :n],
                                             g[:, t, :], start=False,
                                             stop=(done == nmm))
                    # z = dinv * agg (+ b) ; relu
                    if bbc is None:
                        zf = AF.Relu if relu else AF.Copy
                        zt = work.tile([128, F], f32 if emit_out else bf16,
                                       tag="zt%d" % F)
                        nc.scalar.activation(zt[:n, :], pa[:n, :], zf,
                                             scale=dinvs[:n, w:w + 1])
                    else:
                        v = work.tile([128, F], f32, tag="v%d" % F)
                        nc.scalar.activation(v[:n, :], pa[:n, :], AF.Copy,
                                             scale=dinvs[:n, w:w + 1])
                        zt = work.tile([128, F], f32 if emit_out else bf16,
                                       tag="zt%d" % F)
                        if relu:
                            vb = work.tile([128, F], f32, tag="vb%d" % F)
                            nc.vector.tensor_tensor(
                                vb[:n, :], v[:n, :], bbc[:n, :],
                                op=mybir.AluOpType.add)
                            nc.scalar.activation(zt[:n, :], vb[:n, :], AF.Relu)
                        else:
                            nc.vector.tensor_tensor(
                                zt[:n, :], v[:n, :], bbc[:n, :],
                                op=mybir.AluOpType.add)
                    yield w, n, zt

            # ---- phase C: layer-1 aggregation -> h1 -> h1T ----
            for w, n, zt in agg_layer(U1a, U1b, HID, u1res, b1bc, True, False):
                wsl = slice(w * WS, w * WS + n)
                for ch in range(NCH):
                    ptr = psum.tile([128, 128], bf16, tag="tr")
                    nc.tensor.transpose(ptr[:, :n],
                                        zt[:n, ch * 128:(ch + 1) * 128],
                                        ident[:n, :n])
                    nc.scalar.activation(h1T[:, ch, wsl], ptr[:, :n], AF.Copy)
                if MAXP == 3:
                    emit_debug_out(zt[:n, :OUT], w, n)
            if MAXP <= 3:
                return nc

            # ---- phase D: t2 = h1 @ W2 ; u2 ; split AllGather ----
            for j in range(NW):
                n = nsz(j)
                jsl = slice(j * WS, j * WS + n)
                pt = psum.tile([128, OUT], f32, tag="mm")
                for ch in range(NCH):
                    nc.tensor.matmul(pt[:n, :], h1T[:, ch, jsl],
                                     w2s[:, ch, :], start=(ch == 0),
                                     stop=(ch == NCH - 1))
                nc.scalar.activation(u2res[:n, j, :], pt[:n, :], AF.Copy,
                                     scale=dinvs[:n, j:j + 1])
                if MAXP == 4:
                    emit_debug_out(u2res[:n, j, :], j, n)
                if j == SPLITW - 1:
                    store_rng(u2da, u2res, True)
                    nc.gpsimd.collective_compute(
                        "AllGather", mybir.AluOpType.bypass,
                        replica_groups=rg, ins=[u2da[:]], outs=[U2a[:]])
            store_rng(u2db, u2res, False)
            if MAXP <= 4:
                return nc
            nc.gpsimd.collective_compute(
                "AllGather", mybir.AluOpType.bypass, replica_groups=rg,
                ins=[u2db[:]], outs=[U2b[:]])

            # ---- phase F: layer-2 aggregation -> out ----
            for w, n, zt in agg_layer(U2a, U2b, OUT, u2res, b2bc, False, True):
                wsl = slice(w * WS, w * WS + n)
                nc.sync.dma_start(out_p[wsl, :], zt[:n, :])

    return nc


def _assign_gather_queues(nc):
    """Post-schedule queue assignment: queue_num = BIR position % NQ.

    Tile assigns SWDGE DMA-completion semaphore lanes round-robin over the
    *scheduled* order of Pool DMA instructions (lane = pos % 8), ignoring
    queue_num.  Each lane must only ever be incremented by one SWDGE queue
    (decoder shadow-sem rule), so the queue must also be a function of the
    scheduled position: queue = pos % NQ gives queue q the lane set
    {q, q+NQ}.  Build-time round-robin is NOT sufficient because the
    scheduler reorders independent gathers.
    """
    import concourse.mybir as mybir

    pos = 0
    for f in nc.m.functions:
        for bb in f.blocks:
            for inst in bb.instructions:
                if isinstance(inst, mybir.InstDMAGatherAnt):
                    inst.queue_num = pos % NQ
                    pos += 1
                elif (getattr(inst, "engine", None) == mybir.EngineType.Pool
                      and isinstance(inst, (mybir.InstDMACopy,
                                            mybir.InstDMAScatterAddAnt))):
                    raise AssertionError(
                        "unexpected Pool DMA inst would shift SWDGE sem lanes")
    return pos


def run(cfg, inputs, sim=False, trace=False):
    from concourse.bass_utils import run_bass_kernel_spmd

    in_maps, T, b1nz, b2nz = _prepare(
        cfg, inputs["x"], inputs["edge_index"], inputs["W1"], inputs["b1"],
        inputs["W2"], inputs["b2"])
    nc = build_program(cfg, T, b1nz, b2nz)
    nc.finalize()
    _assign_gather_queues(nc)
    core_ids = list(range(cfg.P))
    if sim:
        from concourse import bass_interp
        ms = bass_interp.MultiCoreSim(nc, cfg.P)
        for c in core_ids:
            for k, v in in_maps[c].items():
                ms.cores[c].tensor(k)[:] = v
        ms.simulate()
        outs = [np.array(ms.cores[c].tensor("out")) for c in core_ids]
        return np.concatenate(outs, axis=0), None
    res = run_bass_kernel_spmd(nc, in_maps, core_ids, trace=trace)
    outs = [np.asarray(res.results[c]["out"]) for c in core_ids]
    return np.concatenate(outs, axis=0), res


def kernel(x, edge_index, W1, b1, W2, b2):
    out, _ = run(FULL, dict(x=x, edge_index=edge_index, W1=W1, b1=b1,
                            W2=W2, b2=b2))
    return out



# revision 12
# speedup vs baseline: 1.2681x; 1.2681x over previous
"""GCN encoder (2-layer GCNConv) as a Bass/Tile kernel on 8 Trainium2 NeuronCores.

Strategy (matches the sharding hint):
  - Nodes row-partitioned across 8 cores (6250 rows each); weights replicated.
  - Symmetric normalization factorized: z = D^-1/2 (A+I) D^-1/2 (x W) + b
    =>  u = dinv * (x W);  agg[d] = u[d] + sum_{e:dst=d} u[src_e];
        z = dinv * agg + b
  - Per layer: local matmul -> row scale -> AllGather(u) -> per-core gather of
    source rows (SWDGE dma_gather) -> segment-sum via tensor-engine matmuls
    with 0/1 selection matrices generated on DVE (is_equal vs iota) ->
    scale/bias/relu.
  - Edges bucketed host-side by (dst window of 128, src range half) and padded
    to 128-slot tiles; padded slots gather row 0 and have an all-zero
    selection column, so they contribute nothing.

Gather-pipeline model (what sets the shape of this kernel):
  - SWDGE descriptor generation runs at ~9.3 ns/row on one Q7 pair; 4 queues
    (ucode MAX_SWDGE_QUEUES=4) -> ~2.3 ns/row aggregate floor over ~227k
    gathered rows/core for both layers (~530us).  Everything else (Tensor
    ~330us, HBM drain ~120MB) must hide under it.
  - Tile tracks gather DMA completion on 8 semaphore lanes (lane = scheduled
    position % 8); the gather 8 positions later must WAIT for that completion
    before its own desc-gen (cumulative-threshold lanes must stay ordered).
    So at most 8 gather DMAs are in flight, and throughput is ALSO capped by
    8 * rows_per_call / completion_latency.  Small calls throttle hard (392
    subcalls of ~600 rows ran at ~5 ns/row = 1.1ms total); calls must be BIG.
  - But the SWDGE descriptor ring holds only 128 descs/queue/direction, and a
    call needs rows/16+1 descs; await_space blocks the whole Pool sequencer
    (all queues) when the next call doesn't fit.  Calls of ~9 tiles (73
    descs) stall the sequencer every other call (the 1.04ms baseline's
    pattern).  The sweet spot: calls of <=7 tiles (57 descs) -- two fit per
    ring, desc-gen of call k+1 overlaps the drain of call k, and the lane lap
    (gen+drain+prop ~15us) stays under 2x gen.
  => Host packs each src-half's tile stream h-major (all half-0 groups in
    window order, then half-1) and cuts calls every 7 tiles IGNORING window
    boundaries; a call's tiles can span windows (each tile still belongs to
    one window's selection matmul chain).
  - Per window the aggregation uses TWO psum accumulators: pa0 = self-loop +
    half-0 tiles, drained to SBUF bf16 (z0) as soon as its last half-0 tile
    lands; pa1 = half-1 tiles with z0 re-injected via an identity matmul.
    This lets ALL half-0 gathers (needing only the U*a AllGather) run LAG
    windows ahead of half-1 gathers (needing U*b): no head-of-line blocking
    on the in-order GpSimd queue while the second collective is in flight.
    PSUM pool slots are bank-granular (8 banks): pa0 x3 + pa1 x2 + trd x3.
  - Layer-2's transform (h1 @ W2) is fused per-window into the layer-1 drain
    path (transpose -> matmul -> scale), so the U2a AllGather fires mid-
    layer-1 and layer-2 half-0 gathers are ready when layer-1 ends.
  - Phase A streams x in 4-window blocks on the Sync HWDGE queue; idx/
    selector constants load on the Scalar HWDGE queue; U1a fires after the
    first SPLITW windows.
  - dma_gather queue_num is assigned AFTER scheduling (BIR position % NQ):
    Tile's completion lanes follow scheduled order and a lane must only ever
    be incremented by one queue (decoder shadow-sem rule).
  - Slot padding gathers row 0 (index 0): the SWDGE decoder reserves ring
    space from num_idxs_reg while the Q7 generator trims trailing -1 indices,
    so a -1-padded call leaks ring space and eventually deadlocks await_space.
  - Gather indices are sorted by source row inside each (window, half) group
    (selection columns permuted to match) for ascending-address DMA reads.
"""

import math
import os
import sys

import numpy as np

sys.path.insert(0, "/opt/trn_rl_repo")

import ml_dtypes

BF16 = ml_dtypes.bfloat16

NQ = int(os.environ.get("GCN_NQ", "4"))          # SWDGE queues, round-robin
LAG = int(os.environ.get("GCN_LAG", "20"))       # half-1 window lag
CALL_T = int(os.environ.get("GCN_CALL_T", "7"))  # tiles per gather call


class Cfg:
    def __init__(self, N, E, IN=512, HID=256, OUT=128, P=8):
        self.N, self.E, self.IN, self.HID, self.OUT, self.P = N, E, IN, HID, OUT, P
        self.NC = N // P                      # nodes per core
        self.WS = 128                         # dst window size
        self.NW = math.ceil(self.NC / self.WS)  # windows per core
        # split local rows into two window-aligned ranges; each range's
        # cross-rank concat must stay int16-addressable (< 32768 rows)
        self.SPLITW = (self.NW + 1) // 2
        self.NCA = min(self.SPLITW * self.WS, self.NC)
        self.NCB = self.NC - self.NCA
        assert P * self.NCA <= 32767 and P * self.NCB <= 32767


FULL = Cfg(N=50000, E=800000)


class Plan:
    """Shared (core-independent) gather layout: h-major tile streams."""

    def __init__(self, T):
        self.T = T
        NW = T.shape[0]
        self.owner = {}     # h -> window of each tile in the h stream
        self.gstart = {}    # h -> per-window first tile index in the h stream
        self.calls = {}     # h -> [(tile_lo, ntiles)]
        self.hbase = {}     # h -> tile offset of the h stream in the layout
        base = 0
        for h in (0, 1):
            own = []
            gst = np.zeros(NW, dtype=np.int64)
            for w in range(NW):
                gst[w] = len(own)
                own.extend([w] * int(T[w, h]))
            self.owner[h] = own
            self.gstart[h] = gst
            self.calls[h] = [(lo, min(CALL_T, len(own) - lo))
                             for lo in range(0, len(own), CALL_T)]
            self.hbase[h] = base
            base += len(own)
        self.tiles_total = base


def _prepare(cfg, x, edge_index, W1, b1, W2, b2):
    """Host-side graph preprocessing -> per-core input maps + program params."""
    N, P, NC, WS, NW = cfg.N, cfg.P, cfg.NC, cfg.WS, cfg.NW
    NCA, NCB = cfg.NCA, cfg.NCB
    src = np.asarray(edge_index[0], dtype=np.int64)
    dst = np.asarray(edge_index[1], dtype=np.int64)

    deg = np.bincount(dst, minlength=N).astype(np.float64) + 1.0  # + self loop
    dinv = (1.0 / np.sqrt(deg)).astype(np.float32)

    # src row ids inside the two all-gathered range tensors
    s_own = src // NC
    s_loc = src % NC
    half = (s_loc >= NCA).astype(np.int64)
    s_row = np.where(half == 0, s_own * NCA + s_loc,
                     s_own * NCB + (s_loc - NCA))

    # group id: ((core, window), src-range) ; groups contiguous after sort
    win_id = (dst // NC) * NW + (dst % NC) // WS
    comp = win_id * 2 + half
    order = np.argsort(comp, kind="stable")
    r_s, d_s, c_s = s_row[order], dst[order], comp[order]
    counts = np.bincount(c_s, minlength=P * NW * 2).reshape(P, NW, 2)

    # shared tile counts per (window, range): max over cores
    T = np.ceil(counts.max(axis=0) / 128).astype(np.int64)  # [NW, 2]
    plan = Plan(T)
    slots_total = plan.tiles_total * 128

    starts = np.zeros(P * NW * 2 + 1, dtype=np.int64)
    np.cumsum(counts.reshape(-1), out=starts[1:])

    in_maps = []
    for c in range(P):
        idx_arr = np.zeros(slots_total, dtype=np.int16)
        aco_arr = np.full(slots_total, -1, dtype=np.float32)
        for h in (0, 1):
            for w in range(NW):
                t_wh = int(T[w, h])
                if t_wh == 0:
                    continue
                g = (c * NW + w) * 2 + h
                n = counts[c, w, h]
                sl = slice(starts[g], starts[g] + n)
                rows = r_s[sl]
                cols = d_s[sl] - c * NC - w * WS
                o = np.argsort(rows, kind="stable")  # ascending DMA reads
                off = (plan.hbase[h] + plan.gstart[h][w]) * 128
                idx_arr[off:off + n] = rows[o].astype(np.int16)
                aco_arr[off:off + n] = cols[o].astype(np.float32)

        dloc = np.concatenate(
            [dinv[c * NC:(c + 1) * NC],
             np.ones(NW * WS - NC, dtype=np.float32)])

        m = {
            "xT": np.ascontiguousarray(
                np.asarray(x[c * NC:(c + 1) * NC], np.float32).astype(BF16).T
                .reshape(cfg.IN // 128, 128, NC).transpose(1, 0, 2)),
            "w1": np.ascontiguousarray(
                np.asarray(W1, np.float32).astype(BF16)
                .reshape(cfg.IN // 128, 128, cfg.HID).transpose(1, 0, 2)),
            "w2": np.ascontiguousarray(
                np.asarray(W2, np.float32).astype(BF16)
                .reshape(cfg.HID // 128, 128, cfg.OUT).transpose(1, 0, 2)),
            "dinvc": np.ascontiguousarray(dloc.reshape(NW, WS).T),
            "idx": np.ascontiguousarray(np.tile(idx_arr.reshape(-1, 16).T, (8, 1))),
            "acol": np.ascontiguousarray(aco_arr.reshape(-1, 128).T.astype(BF16)),
            "ident": np.eye(128, dtype=BF16),
        }
        b1nz = bool(np.any(np.asarray(b1)))
        b2nz = bool(np.any(np.asarray(b2)))
        if b1nz:
            m["b1bc"] = np.ascontiguousarray(
                np.broadcast_to(np.asarray(b1, np.float32), (128, cfg.HID)))
        if b2nz:
            m["b2bc"] = np.ascontiguousarray(
                np.broadcast_to(np.asarray(b2, np.float32), (128, cfg.OUT)))
        in_maps.append(m)

    return in_maps, plan, b1nz, b2nz


def build_program(cfg, plan, b1nz, b2nz):
    import concourse.bacc as bacc
    import concourse.mybir as mybir
    from concourse import tile

    T = plan.T
    N, P, NC, WS, NW = cfg.N, cfg.P, cfg.NC, cfg.WS, cfg.NW
    NCA, NCB, SPLITW = cfg.NCA, cfg.NCB, cfg.SPLITW
    IN, HID, OUT = cfg.IN, cfg.HID, cfg.OUT
    NCI, NCH = IN // 128, HID // 128
    tiles_total = plan.tiles_total
    slots_total = tiles_total * 128
    TMAX = int(T.max())
    f32, bf16, i16 = mybir.dt.float32, mybir.dt.bfloat16, mybir.dt.int16
    f8 = mybir.dt.float8e4
    AF = mybir.ActivationFunctionType
    LAGW = min(LAG, NW)

    nc = bacc.Bacc("TRN2", target_bir_lowering=False, debug=False,
                   num_devices=cfg.P, num_swdge_queues=NQ)
    xT_p = nc.dram_tensor("xT", [128, NCI, NC], bf16, kind="ExternalInput")
    w1_p = nc.dram_tensor("w1", [128, NCI, HID], bf16, kind="ExternalInput")
    w2_p = nc.dram_tensor("w2", [128, NCH, OUT], bf16, kind="ExternalInput")
    dinv_p = nc.dram_tensor("dinvc", [WS, NW], f32, kind="ExternalInput")
    idx_p = nc.dram_tensor("idx", [128, slots_total // 16], i16, kind="ExternalInput")
    acol_p = nc.dram_tensor("acol", [128, tiles_total], bf16, kind="ExternalInput")
    id_p = nc.dram_tensor("ident", [128, 128], bf16, kind="ExternalInput")
    b1_p = (nc.dram_tensor("b1bc", [128, HID], f32, kind="ExternalInput")
            if b1nz else None)
    b2_p = (nc.dram_tensor("b2bc", [128, OUT], f32, kind="ExternalInput")
            if b2nz else None)
    out_p = nc.dram_tensor("out", [NC, OUT], f32, kind="ExternalOutput")

    u1da = nc.dram_tensor("u1da", [NCA, HID], f8)
    u1db = nc.dram_tensor("u1db", [NCB, HID], f8)
    u2da = nc.dram_tensor("u2da", [NCA, OUT], bf16)
    u2db = nc.dram_tensor("u2db", [NCB, OUT], bf16)
    U1a = nc.dram_tensor("U1a", [P * NCA, HID], f8, addr_space="Shared")
    U1b = nc.dram_tensor("U1b", [P * NCB, HID], f8, addr_space="Shared")
    U2a = nc.dram_tensor("U2a", [P * NCA, OUT], bf16, addr_space="Shared")
    U2b = nc.dram_tensor("U2b", [P * NCB, OUT], bf16, addr_space="Shared")
    rg = [list(range(P))]

    with tile.TileContext(nc) as tc:
        with (
            tc.tile_pool(name="res", bufs=1) as res,
            tc.tile_pool(name="work", bufs=4) as work,
            tc.tile_pool(name="gath", bufs=12) as gath,
            tc.tile_pool(name="psum", bufs=2, space="PSUM") as psum,
        ):
            # ---- resident loads (scalar HWDGE queue; sync reserved for x) ----
            idxs = res.tile([128, slots_total // 16], i16)
            nc.scalar.dma_start(idxs[:], idx_p[:])
            acols = res.tile([128, tiles_total], bf16)
            nc.scalar.dma_start(acols[:], acol_p[:])
            w1s = res.tile([128, NCI, HID], bf16)
            nc.scalar.dma_start(w1s[:], w1_p[:])
            w2s = res.tile([128, NCH, OUT], bf16)
            nc.scalar.dma_start(w2s[:], w2_p[:])
            dinvs = res.tile([WS, NW], f32)
            nc.scalar.dma_start(dinvs[:], dinv_p[:])
            ident = res.tile([128, 128], bf16)
            nc.scalar.dma_start(ident[:], id_p[:])
            iot = res.tile([128, TMAX, 128], bf16)
            nc.gpsimd.iota(iot[:], pattern=[[0, TMAX], [1, 128]], base=0,
                           channel_multiplier=0,
                           allow_small_or_imprecise_dtypes=True)
            b1bc = None
            if b1nz:
                b1bc = res.tile([128, HID], f32)
                nc.scalar.dma_start(b1bc[:], b1_p[:])
            b2bc = None
            if b2nz:
                b2bc = res.tile([128, OUT], f32)
                nc.scalar.dma_start(b2bc[:], b2_p[:])

            cnt_regs = {}
            for h in (0, 1):
                for _, nt in plan.calls[h]:
                    if nt not in cnt_regs:
                        cnt_regs[nt] = nc.gpsimd.to_reg(128 * nt)

            u1res = res.tile([128, NW, HID], bf16)
            u1f8 = res.tile([128, NW, HID], f8)
            u2res = res.tile([128, NW, OUT], bf16)
            h1T = res.tile([128, NCH, NC], bf16)
            if NC % WS:
                # tail rows of the last window feed matmuls as rhs; zero them
                # so uninitialized SBUF can't inject NaNs
                nc.gpsimd.memset(u1res[:, NW - 1, :], 0.0)
                nc.gpsimd.memset(u1f8[:, NW - 1, :], 0.0)
                nc.gpsimd.memset(u2res[:, NW - 1, :], 0.0)

            def nsz(j):
                return min(128, NC - j * WS)

            def store_rng(ud, ures, rng_a):
                """Batched store of a window range of ures into ud."""
                if rng_a:
                    w0, rows = 0, NCA
                else:
                    w0, rows = SPLITW, NCB
                nfull = rows // WS
                tail = rows - nfull * WS
                if nfull:
                    dstp = ud[0:nfull * WS, :].rearrange(
                        "(w p) f -> p w f", p=WS)
                    nc.sync.dma_start(dstp, ures[:, w0:w0 + nfull, :])
                if tail:
                    nc.sync.dma_start(ud[nfull * WS:, :],
                                      ures[:tail, w0 + nfull, :])

            # ---- phase A: u1 = dinv * (x @ W1); split AllGather ----
            XB = 4  # windows per x block
            blocks = []
            for lo in range(0, SPLITW, XB):
                blocks.append((lo, min(lo + XB, SPLITW)))
            for lo in range(SPLITW, NW, XB):
                blocks.append((lo, min(lo + XB, NW)))
            for (lo, hi) in blocks:
                cols = min(hi * WS, NC) - lo * WS
                xb = work.tile([128, NCI, XB * WS], bf16, tag="xb", bufs=2)
                nc.sync.dma_start(xb[:, :, :cols],
                                  xT_p[:, :, lo * WS:lo * WS + cols])
                for w in range(lo, hi):
                    n = nsz(w)
                    o = (w - lo) * WS
                    pt = psum.tile([128, HID], f32, tag="pa0", bufs=3)
                    for ci in range(NCI):
                        nc.tensor.matmul(pt[:n, :], xb[:, ci, o:o + n],
                                         w1s[:, ci, :], start=(ci == 0),
                                         stop=(ci == NCI - 1))
                    nc.scalar.activation(u1res[:n, w, :], pt[:n, :], AF.Copy,
                                         scale=dinvs[:n, w:w + 1])
                    nc.scalar.activation(u1f8[:n, w, :], pt[:n, :], AF.Copy,
                                         scale=dinvs[:n, w:w + 1])
                if hi == SPLITW:
                    store_rng(u1da, u1f8, True)
                    nc.gpsimd.collective_compute(
                        "AllGather", mybir.AluOpType.bypass,
                        replica_groups=rg, ins=[u1da[:]], outs=[U1a[:]])
            store_rng(u1db, u1f8, False)
            nc.gpsimd.collective_compute(
                "AllGather", mybir.AluOpType.bypass, replica_groups=rg,
                ins=[u1db[:]], outs=[U1b[:]])

            zsave = [None] * NW

            def layer(Ua, Ub, F, mdt, ures, zsink):
                """One aggregation layer: packed gather calls, lagged half-1."""
                st = {0: {}, 1: {}}  # h -> w -> dict(S=, pa=, left=)

                def emit_call(h, lo, nt, Uh):
                    g = gath.tile([128, CALL_T, F], mdt, tag="g%d" % F,
                                  bufs=12)
                    so = (plan.hbase[h] + lo) * 128
                    nc.gpsimd.dma_gather(
                        g[:, :nt, :], Uh[:],
                        idxs[:, so // 16:(so + 128 * nt) // 16],
                        num_idxs=128 * nt, num_idxs_reg=cnt_regs[nt],
                        elem_size=F,
                        single_packet=os.environ.get("GCN_SP", "1") == "1",
                        queue_num=0)
                    for t in range(nt):
                        w = plan.owner[h][lo + t]
                        n = nsz(w)
                        s = st[h].get(w)
                        if s is None:
                            t_wh = int(T[w, h])
                            S = work.tile([128, TMAX, 128], mdt,
                                          tag="S", bufs=6)
                            gt = plan.hbase[h] + plan.gstart[h][w]
                            nc.vector.tensor_tensor(
                                S[:, :t_wh, :], iot[:, :t_wh, :],
                                acols[:, gt:gt + t_wh]
                                .broadcast_to((128, t_wh, 128)),
                                op=mybir.AluOpType.is_equal)
                            tag = "pa0" if h == 0 else "pa1"
                            pa = psum.tile([128, F], f32, tag=tag,
                                           bufs=3 if h == 0 else 2)
                            rhs = ures[:, w, :] if h == 0 else zsave[w][:, :]
                            nc.tensor.matmul(pa[:n, :], ident[:, :n], rhs,
                                             start=True, stop=False)
                            s = {"S": S, "pa": pa, "left": t_wh, "lt": 0}
                            st[h][w] = s
                        lt = s["lt"]
                        s["lt"] += 1
                        s["left"] -= 1
                        last = s["left"] == 0
                        nc.tensor.matmul(s["pa"][:n, :], s["S"][:, lt, :n],
                                         g[:, t, :], start=False, stop=last)
                        if last:
                            if h == 0:
                                z0 = work.tile([128, F], bf16, tag="z0",
                                               bufs=LAGW + 3)
                                nc.scalar.activation(z0[:n, :],
                                                     s["pa"][:n, :], AF.Copy)
                                zsave[w] = z0
                            else:
                                zsink(w, n, s["pa"])
                                zsave[w] = None
                            del st[h][w]

                h1_calls = plan.calls[1]
                i1 = 0
                w0done = -1
                for (lo, nt) in plan.calls[0]:
                    emit_call(0, lo, nt, Ua)
                    w0done = plan.owner[0][lo + nt - 1]
                    if lo + nt == len(plan.owner[0]):
                        w0done = NW - 1
                    else:
                        w0done -= 1  # last window may be incomplete
                    while (i1 < len(h1_calls)
                           and plan.owner[1][h1_calls[i1][0]
                                             + h1_calls[i1][1] - 1] + LAGW
                           <= w0done):
                        emit_call(1, h1_calls[i1][0], h1_calls[i1][1], Ub)
                        i1 += 1
                while i1 < len(h1_calls):
                    emit_call(1, h1_calls[i1][0], h1_calls[i1][1], Ub)
                    i1 += 1

            # layer-1 sink: scale/relu, transpose -> h1T, fused phase-D
            # matmul, u2 scale, and the U2a collective at the range-a boundary
            def l1_sink(w, n, pa):
                wsl = slice(w * WS, w * WS + n)
                zt = work.tile([128, HID], bf16, tag="zt", bufs=4)
                nc.scalar.activation(zt[:n, :], pa[:n, :], AF.Relu,
                                     scale=dinvs[:n, w:w + 1])
                for ch in range(NCH):
                    ptr = psum.tile([128, 128], bf16, tag="trd", bufs=3)
                    nc.tensor.transpose(ptr[:, :n],
                                        zt[:n, ch * 128:(ch + 1) * 128],
                                        ident[:n, :n])
                    nc.scalar.activation(h1T[:, ch, wsl], ptr[:, :n], AF.Copy)
                pt2 = psum.tile([128, OUT], f32, tag="trd", bufs=3)
                for ch in range(NCH):
                    nc.tensor.matmul(pt2[:n, :], h1T[:, ch, wsl],
                                     w2s[:, ch, :], start=(ch == 0),
                                     stop=(ch == NCH - 1))
                nc.scalar.activation(u2res[:n, w, :], pt2[:n, :], AF.Copy,
                                     scale=dinvs[:n, w:w + 1])
                if w == SPLITW - 1:
                    store_rng(u2da, u2res, True)
                    nc.gpsimd.collective_compute(
                        "AllGather", mybir.AluOpType.bypass,
                        replica_groups=rg, ins=[u2da[:]], outs=[U2a[:]])

            layer(U1a, U1b, HID, f8, u1res, l1_sink)
            store_rng(u2db, u2res, False)
            nc.gpsimd.collective_compute(
                "AllGather", mybir.AluOpType.bypass, replica_groups=rg,
                ins=[u2db[:]], outs=[U2b[:]])

            # layer-2 sink: scale (+bias) and store output rows
            def l2_sink(w, n, pa):
                zt = work.tile([128, OUT], f32, tag="zt2", bufs=4)
                if b2bc is None:
                    nc.scalar.activation(zt[:n, :], pa[:n, :], AF.Copy,
                                         scale=dinvs[:n, w:w + 1])
                else:
                    v = work.tile([128, OUT], f32, tag="v", bufs=2)
                    nc.scalar.activation(v[:n, :], pa[:n, :], AF.Copy,
                                         scale=dinvs[:n, w:w + 1])
                    nc.vector.tensor_tensor(zt[:n, :], v[:n, :], b2bc[:n, :],
                                            op=mybir.AluOpType.add)
                nc.sync.dma_start(out_p[w * WS:w * WS + n, :], zt[:n, :])

            layer(U2a, U2b, OUT, bf16, u2res, l2_sink)

    return nc


def _assign_gather_queues(nc):
    """Post-schedule queue assignment: queue_num = BIR position % NQ.

    Tile assigns SWDGE DMA-completion semaphore lanes round-robin over the
    *scheduled* order of Pool DMA instructions (lane = pos % 8), ignoring
    queue_num.  Each lane must only ever be incremented by one SWDGE queue
    (decoder shadow-sem rule), so the queue must also be a function of the
    scheduled position: queue = pos % NQ gives queue q the lane set
    {q, q+NQ}.  Build-time round-robin is NOT sufficient because the
    scheduler reorders independent gathers.
    """
    import concourse.mybir as mybir

    pos = 0
    for f in nc.m.functions:
        for bb in f.blocks:
            for inst in bb.instructions:
                if isinstance(inst, mybir.InstDMAGatherAnt):
                    inst.queue_num = pos % NQ
                    pos += 1
                elif (getattr(inst, "engine", None) == mybir.EngineType.Pool
                      and isinstance(inst, (mybir.InstDMACopy,
                                            mybir.InstDMAScatterAddAnt))):
                    raise AssertionError(
                        "unexpected Pool DMA inst would shift SWDGE sem lanes")
    return pos


def run(cfg, inputs, sim=False, trace=False):
    from concourse.bass_utils import run_bass_kernel_spmd

    in_maps, plan, b1nz, b2nz = _prepare(
        cfg, inputs["x"], inputs["edge_index"], inputs["W1"], inputs["b1"],
        inputs["W2"], inputs["b2"])
    nc = build_program(cfg, plan, b1nz, b2nz)
    nc.finalize()
    _assign_gather_queues(nc)
    core_ids = list(range(cfg.P))
    if sim:
        from concourse import bass_interp
        ms = bass_interp.MultiCoreSim(nc, cfg.P)
        for c in core_ids:
            for k, v in in_maps[c].items():
                ms.cores[c].tensor(k)[:] = v
        ms.simulate()
        outs = [np.array(ms.cores[c].tensor("out")) for c in core_ids]
        return np.concatenate(outs, axis=0), None
    res = run_bass_kernel_spmd(nc, in_maps, core_ids, trace=trace)
    outs = [np.asarray(res.results[c]["out"]) for c in core_ids]
    return np.concatenate(outs, axis=0), res


def kernel(x, edge_index, W1, b1, W2, b2):
    out, _ = run(FULL, dict(x=x, edge_index=edge_index, W1=W1, b1=b1,
                            W2=W2, b2=b2))
    return out
